# revision 7
# baseline (speedup 1.0000x reference)
"""Fused cross-modal attention (concat two QKV streams along sequence, full
softmax attention) on 8 Trainium2 NeuronCores.

Sharding: data-parallel over (batch b, modality-half h) -> 8 shards. Each core
computes attention for 2048 queries against the fused 4096-key sequence.

Host-side prep (per core): Q and K are pre-transposed to d-major [64, n] and
pre-permuted (column j*128+p <-> row p*T+j) so the device needs no PE
transposes; V gets its ones-column appended ([V | 1], key-permuted to match
K's column order) so the PV matmul yields the softmax denominator for free.
All three are cast to bf16 on host: matmul streams at full 2.4 GHz PE clock
(fp32r runs at the half-rate clock) and input DMA bytes are halved.

Per-core kernel (all operands resident in SBUF):
  - Software-pipelined main loop over 2 query-halves x 32 key-tiles:
      scoresT[k=128, q=1024]  = kt_chunk.T @ qt_block     (PSUM, 2 matmuls)
      expS = exp(scale*scoresT) -> bf16                   (one ACT instr)
      acc[65, 1024]          += [V | 1].T @ expS          (PSUM accumulate)
    QK(s+1) is emitted BEFORE PV(s) so the in-order PE computes next-tile
    scores while ACT runs exp(s).
  - Epilogue per query-half: copy acc to SBUF (frees PSUM for the next
    half), PE-transpose 128-query chunks to [128, 65], then per-partition
    reciprocal of the denominator column + tensor_scalar multiply, and DMA
    straight to the [2048, 64] output with an access pattern that undoes
    the query permutation (each partition's 8 chunks are contiguous rows).
"""

import numpy as np
import ml_dtypes

import concourse.bass as bass
import concourse.tile as tile
from concourse import mybir
from concourse.bacc import Bacc
from concourse.bass_utils import run_bass_kernel_spmd
from concourse.masks import make_identity

F32 = mybir.dt.float32
BF16 = mybir.dt.bfloat16
NPBF16 = ml_dtypes.bfloat16

B, S, D = 4, 2048, 64
S2 = 2 * S  # fused sequence length 4096
NCORES = 8
QSH = 2048  # queries per core (= S: half of the fused sequence)
KT = S2 // 128  # 32 key tiles of 128
QT = QSH // 128  # 16 query tiles of 128
SCALE = 1.0 / float(np.sqrt(D))


def _build():
    nc = Bacc()
    # qt[d, j*128+p] = Q[p*16+j, d]; kt[d, i*128+p] = K[p*32+i, d]
    qt_d = nc.declare_dram_parameter("qt", [D, QSH], BF16, isOutput=False)
    kt_d = nc.declare_dram_parameter("kt", [D, S2], BF16, isOutput=False)
    # v1[p, i, 0:64] = V[p*32+i, :], v1[p, i, 64] = 1.0
    v1_d = nc.declare_dram_parameter("v1", [128, KT * (D + 1)], BF16, isOutput=False)
    out = nc.declare_dram_parameter("out", [QSH, D], F32, isOutput=True)

    with tile.TileContext(nc) as tc:
        with (
            tc.tile_pool(name="const", bufs=1) as const_pool,
            tc.tile_pool(name="stage", bufs=1) as stage,
            tc.tile_pool(name="psum", bufs=2, space="PSUM") as psum,
            tc.tile_pool(name="tpp", bufs=2, space="PSUM") as tpp,
            tc.tile_pool(name="apsum", bufs=1, space="PSUM") as apsum,
            tc.tile_pool(name="exps", bufs=3) as exps,
            tc.tile_pool(name="outp", bufs=3) as outp,
        ):
            ident = const_pool.tile([128, 128], F32)
            make_identity(nc, ident)
            # Touch Exp early so the ~1.3us ACT table load overlaps the
            # input DMAs instead of stalling the first real exp.
            warm = const_pool.tile([128, 1], F32)
            nc.scalar.activation(
                out=warm, in_=ident[:, 0:1],
                func=mybir.ActivationFunctionType.Exp,
            )

            # out[p*16 + h*8 + t, :] <- half h, chunk t, partition p;
            # the 8 chunks of one half are contiguous rows per partition.
            out_ap = out[:].rearrange("(p g t) d -> g p (t d)", g=2, t=8)
            v1_ap = v1_d[:]

            # Chunked staging loads ordered so the first main-loop
            # iteration's operands land first.
            NKC = 8  # key tiles per kt chunk
            kt_chunks = [
                stage.tile([D, NKC * 128], BF16, name=f"kt{c}", tag=f"kt{c}")
                for c in range(KT // NKC)
            ]
            qt_chunks = [
                stage.tile([D, 1024], BF16, name=f"qt{c}", tag=f"qt{c}")
                for c in range(2)
            ]
            v1_chunks = [
                stage.tile([128, KT // 2, D + 1], BF16, name=f"v1{c}", tag=f"v1{c}")
                for c in range(2)
            ]
            nc.sync.dma_start(out=kt_chunks[0], in_=kt_d[:, 0 : NKC * 128])
            nc.sync.dma_start(out=qt_chunks[0], in_=qt_d[:, 0:1024])
            nc.sync.dma_start(out=v1_chunks[0], in_=v1_ap[:, 0 : (KT // 2) * (D + 1)])
            for c in range(1, KT // NKC):
                nc.sync.dma_start(
                    out=kt_chunks[c], in_=kt_d[:, c * NKC * 128 : (c + 1) * NKC * 128]
                )
            nc.sync.dma_start(out=qt_chunks[1], in_=qt_d[:, 1024:2048])
            nc.sync.dma_start(
                out=v1_chunks[1], in_=v1_ap[:, (KT // 2) * (D + 1) : KT * (D + 1)]
            )

            # Software-pipelined main loop over 64 (half, key-tile) steps.
            NS = 2 * KT  # 64 pipeline steps; step s = (h, i) = divmod(s, KT)

            def qk(s):
                h, i = divmod(s, KT)
                kt_blk = kt_chunks[i // NKC][:, (i % NKC) * 128 : (i % NKC + 1) * 128]
                sc = psum.tile([128, 1024], F32, tag="sc", name=f"sc{s}")
                for j in range(2):
                    nc.tensor.matmul(
                        sc[:, j * 512 : (j + 1) * 512],
                        lhsT=kt_blk,
                        rhs=qt_chunks[h][:, j * 512 : (j + 1) * 512],
                        start=True,
                        stop=True,
                    )
                return sc

            def pv(s, ex, acc):
                h, i = divmod(s, KT)
                v1_blk = v1_chunks[i // (KT // 2)][:, i % (KT // 2), :]
                for j in range(2):
                    nc.tensor.matmul(
                        acc[:, j * 512 : (j + 1) * 512],
                        lhsT=v1_blk,
                        rhs=ex[:, j * 512 : (j + 1) * 512],
                        start=(i == 0),
                        stop=(i == KT - 1),
                        skip_group_check=True,
                    )

            # Deferred epilogue work: list of (half, chunk) still to transpose.
            epi_queue = []

            def epilogue_chunk(h, t, acc_sb, ot):
                tr = tpp.tile([128, 128], F32, tag="tp", name=f"tr{h}_{t}")
                nc.tensor.transpose(
                    tr[:, 0:65],
                    acc_sb[:, t * 128 : (t + 1) * 128],
                    ident[0:65, 0:65],
                )
                rc = outp.tile([128, 1], F32, tag="rc", name=f"rc{h}_{t}")
                nc.vector.reciprocal(rc, tr[:, 64:65])
                nc.vector.tensor_scalar_mul(ot[:, t, :], tr[:, 0:D], rc)

            accs = [None, None]
            epi = [None, None]  # (acc_sb, ot) per half
            sc_cur = qk(0)
            for s in range(NS):
                h, i = divmod(s, KT)
                if i == 0:
                    accs[h] = apsum.tile([65, 1024], F32, name=f"acc{h}", tag="acc")
                ex = exps.tile([128, 1024], BF16, name=f"ex{s % 4}", tag="ex")
                nc.scalar.activation(
                    out=ex,
                    in_=sc_cur,
                    func=mybir.ActivationFunctionType.Exp,
                    scale=SCALE,
                )
                if s + 1 < NS:
                    sc_cur = qk(s + 1)
                pv(s, ex, accs[h])
                # Drain one deferred h=0 epilogue chunk per step while the
                # h=1 main loop keeps the PE/ACT pipeline full.
                if epi_queue and s >= KT + 1:
                    eh, t = epi_queue.pop(0)
                    epilogue_chunk(eh, t, *epi[eh])
                    if eh == 0 and t == 3:
                        nc.sync.dma_start(
                            out=out_ap[0, :, 0 : 4 * D], in_=epi[0][1][:, 0:4, :]
                        )
                    if eh == 0 and t == 7:
                        nc.sync.dma_start(
                            out=out_ap[0, :, 4 * D : 8 * D], in_=epi[0][1][:, 4:8, :]
                        )
                if i == KT - 1:
                    # Accumulation for this half just finished: move it to
                    # SBUF (frees PSUM banks for the other half's acc).
                    acc_sb = outp.tile(
                        [65, 1024], F32, tag=f"acc_sb{h}", name=f"acc_sb{h}"
                    )
                    for part in range(2):
                        nc.vector.tensor_copy(
                            out=acc_sb[:, part * 512 : (part + 1) * 512],
                            in_=accs[h][:, part * 512 : (part + 1) * 512],
                        )
                    ot = outp.tile([128, 8, D], F32, tag=f"ot{h}", name=f"ot{h}")
                    epi[h] = (acc_sb, ot)
                    epi_queue.extend((h, t) for t in range(8))

            # Tail: h=1 epilogue. Interleave output DMAs (on both HWDGE
            # queues: SP and ACT) with the remaining transpose/normalize.
            for eh, t in epi_queue:
                epilogue_chunk(eh, t, *epi[eh])
                if eh == 1 and t == 3:
                    nc.sync.dma_start(
                        out=out_ap[1, :, 0 : 4 * D], in_=epi[1][1][:, 0:4, :]
                    )
            nc.scalar.dma_start(out=out_ap[1, :, 4 * D : 8 * D], in_=epi[1][1][:, 4:8, :])

    nc.finalize()
    return nc


_NC = None


def _get_nc():
    global _NC
    if _NC is None:
        _NC = _build()
    return _NC


def _dmajor(x, tiles):
    """[n, D] row-major -> [D, n] with column j*128+p <-> row p*tiles+j."""
    return np.ascontiguousarray(
        x.reshape(128, tiles, D).transpose(2, 1, 0).reshape(D, 128 * tiles)
    ).astype(NPBF16)


def _shard_inputs(Q1, K1, V1, Q2, K2, V2):
    """Core c handles batch c//2, modality-half c%2."""
    in_maps = []
    for c in range(NCORES):
        b, h = divmod(c, 2)
        qs = Q1[b] if h == 0 else Q2[b]
        ks = np.concatenate([K1[b], K2[b]], axis=0)
        vs = np.concatenate([V1[b], V2[b]], axis=0)
        v1 = np.ones((128, KT, D + 1), dtype=np.float32)
        v1[:, :, 0:D] = vs.reshape(128, KT, D)
        in_maps.append(
            {
                "qt": _dmajor(qs, QT),
                "kt": _dmajor(ks, KT),
                "v1": v1.reshape(128, KT * (D + 1)).astype(NPBF16),
            }
        )
    return in_maps


def _assemble(results):
    out = np.empty((B, S2, D), dtype=np.float32)
    for c in range(NCORES):
        b, h = divmod(c, 2)
        out[b, h * QSH : (h + 1) * QSH, :] = results[c]["out"]
    return out


def run(inputs, trace=False):
    nc = _get_nc()
    in_maps = _shard_inputs(
        np.asarray(inputs["Q1"]), np.asarray(inputs["K1"]), np.asarray(inputs["V1"]),
        np.asarray(inputs["Q2"]), np.asarray(inputs["K2"]), np.asarray(inputs["V2"]),
    )
    bkr = run_bass_kernel_spmd(nc, in_maps, list(range(NCORES)), trace=trace)
    return _assemble(bkr.results), bkr


def kernel(**inputs) -> np.ndarray:
    out, _ = run(inputs)
    return out


# revision 10
# speedup vs baseline: 1.3133x; 1.3133x over previous
"""Fused cross-modal attention (concat two QKV streams along sequence, full
softmax attention) on 8 Trainium2 NeuronCores.

Sharding: data-parallel over (batch b, modality-half h) -> 8 shards. Each core
computes attention for 2048 queries against the fused 4096-key sequence.

Host-side prep (per core): Q and K are pre-transposed to d-major and
pre-permuted (column j*128+p <-> row p*T+j) so the device needs no PE
transposes; V gets a ones-column appended ([V | 1], key-permuted to match K)
so the PV matmul yields the softmax denominator for free. All three are cast
to bf16 on host (full 2.4 GHz PE stream rate, half the DMA bytes).

PE row-group alternation: the PE pulls LDWEIGHTS ahead of an in-flight
matmul only when the row groups don't conflict, so every matmul here is a
64-row-contraction tile and consecutive matmuls strictly alternate between
PE row halves A (partitions 0-63) and B (64-127):
  - QK (contraction d=64): kt and qt are staged TWICE, on partitions 0-63
    and 64-127; the j0 matmul runs on half A, j1 on half B.
  - PV (contraction keys=128): split into two K=64 matmuls - V/ex rows 0-63
    on half A into accA, rows 64-127 on half B into accB. Separate PSUM
    accumulators per row half (different row tiles must not hit the same
    PSUM bank); the epilogue sums accA+accB on the DVE.
The whole main loop stays in one 64-row tiling mode (mode switches drain
the PE), so the output transposes run in the tail after the last matmul.

Pipeline per step s=(h,i): exp(s) [ACT] is emitted, then QK(s+1) [PE], then
the four PV(s) matmuls [PE] - the in-order PE computes next-tile scores
while ACT runs exp(s). Output DMA is chunked across both HWDGE queues
(SP + ACT) and issued as chunks are normalized, keeping the final drain
short.
"""

import numpy as np
import ml_dtypes

import concourse.bass as bass
import concourse.tile as tile
from concourse import mybir
from concourse.bacc import Bacc
from concourse.bass_utils import run_bass_kernel_spmd
from concourse.masks import make_identity

F32 = mybir.dt.float32
BF16 = mybir.dt.bfloat16
NPBF16 = ml_dtypes.bfloat16

B, S, D = 4, 2048, 64
S2 = 2 * S  # fused sequence length 4096
NCORES = 8
QSH = 2048  # queries per core (= S: half of the fused sequence)
KT = S2 // 128  # 32 key tiles of 128
QT = QSH // 128  # 16 query tiles of 128
SCALE = 1.0 / float(np.sqrt(D))

# kt chunk ranges (in key-tile units): first chunk small so the first
# matmul's operands land quickly.
KT_CHUNKS = [(0, 2), (2, 8), (8, 16), (16, 24), (24, 32)]
V1_CHUNKS = [(0, 4), (4, 16), (16, 32)]


def _build():
    nc = Bacc()
    # qt[d, j*128+p] = Q[p*16+j, d]; kt[d, i*128+p] = K[p*32+i, d]
    qt_d = nc.declare_dram_parameter("qt", [D, QSH], BF16, isOutput=False)
    kt_d = nc.declare_dram_parameter("kt", [D, S2], BF16, isOutput=False)
    # v1[p, i, 0:64] = V[p*32+i, :], v1[p, i, 64] = 1.0
    v1_d = nc.declare_dram_parameter("v1", [128, KT * (D + 1)], BF16, isOutput=False)
    out = nc.declare_dram_parameter("out", [QSH, D], F32, isOutput=True)

    with tile.TileContext(nc) as tc:
        with (
            tc.tile_pool(name="const", bufs=1) as const_pool,
            tc.tile_pool(name="stage", bufs=1) as stage,
            tc.tile_pool(name="psum", bufs=2, space="PSUM") as psum,
            tc.tile_pool(name="apsum", bufs=1, space="PSUM") as apsum,
            tc.tile_pool(name="exps", bufs=3) as exps,
            tc.tile_pool(name="outp", bufs=3) as outp,
        ):
            ident = const_pool.tile([128, 128], F32)
            make_identity(nc, ident)
            # Touch Exp early so the ~1.3us ACT table load overlaps the
            # input DMAs instead of stalling the first real exp.
            warm = const_pool.tile([128, 1], F32)
            nc.scalar.activation(
                out=warm, in_=ident[:, 0:1],
                func=mybir.ActivationFunctionType.Exp,
            )

            # out[p*16 + h*8 + t, :] <- half h, chunk t, partition p;
            # the 8 chunks of one half are contiguous rows per partition.
            out_ap = out[:].rearrange("(p g t) d -> g p (t d)", g=2, t=8)
            v1_ap = v1_d[:].rearrange("p (t e) -> p t e", e=D + 1)

            # kt/qt staged on BOTH partition halves (row-group alternation).
            kt_tiles = [
                stage.tile([128, (b - a) * 128], BF16, name=f"kt{a}", tag=f"kt{a}")
                for a, b in KT_CHUNKS
            ]
            qt_tiles = [
                stage.tile([128, 1024], BF16, name=f"qt{h}", tag=f"qt{h}")
                for h in range(2)
            ]
            v1_tiles = [
                stage.tile([128, b - a, D + 1], BF16, name=f"v1{a}", tag=f"v1{a}")
                for a, b in V1_CHUNKS
            ]

            def dma_dup(t, src):  # stage src into both partition halves
                nc.sync.dma_start(out=t[0:64, :], in_=src)
                nc.sync.dma_start(out=t[64:128, :], in_=src)

            a, b = KT_CHUNKS[0]
            dma_dup(kt_tiles[0], kt_d[:, a * 128 : b * 128])
            dma_dup(qt_tiles[0], qt_d[:, 0:1024])
            a, b = V1_CHUNKS[0]
            nc.sync.dma_start(out=v1_tiles[0], in_=v1_ap[:, a:b, :])
            a, b = KT_CHUNKS[1]
            dma_dup(kt_tiles[1], kt_d[:, a * 128 : b * 128])
            a, b = V1_CHUNKS[1]
            nc.sync.dma_start(out=v1_tiles[1], in_=v1_ap[:, a:b, :])
            dma_dup(qt_tiles[1], qt_d[:, 1024:2048])
            for c in range(2, len(KT_CHUNKS)):
                a, b = KT_CHUNKS[c]
                dma_dup(kt_tiles[c], kt_d[:, a * 128 : b * 128])
            a, b = V1_CHUNKS[2]
            nc.sync.dma_start(out=v1_tiles[2], in_=v1_ap[:, a:b, :])

            def kt_blk(i, half):
                for c, (a, b) in enumerate(KT_CHUNKS):
                    if a <= i < b:
                        lo = 64 * half
                        return kt_tiles[c][
                            lo : lo + 64, (i - a) * 128 : (i - a + 1) * 128
                        ]
                raise AssertionError

            def v1_blk(i, half):
                for c, (a, b) in enumerate(V1_CHUNKS):
                    if a <= i < b:
                        lo = 64 * half
                        return v1_tiles[c][lo : lo + 64, i - a, :]
                raise AssertionError

            NS = 2 * KT  # 64 pipeline steps; step s = (h, i) = divmod(s, KT)

            def qk(s):
                h, i = divmod(s, KT)
                sc = psum.tile([128, 1024], F32, tag="sc", name=f"sc{s}")
                for j in range(2):  # j0 on row half A, j1 on half B
                    lo = 64 * j
                    nc.tensor.matmul(
                        sc[:, j * 512 : (j + 1) * 512],
                        lhsT=kt_blk(i, j),
                        rhs=qt_tiles[h][lo : lo + 64, j * 512 : (j + 1) * 512],
                        start=True,
                        stop=True,
                    )
                return sc

            def pv(s, ex, acc2):
                h, i = divmod(s, KT)
                for j in range(2):
                    for half in range(2):  # key rows 0-63 on A, 64-127 on B
                        lo = 64 * half
                        nc.tensor.matmul(
                            acc2[half][:, j * 512 : (j + 1) * 512],
                            lhsT=v1_blk(i, half),
                            rhs=ex[lo : lo + 64, j * 512 : (j + 1) * 512],
                            start=(i == 0),
                            stop=(i == KT - 1),
                            skip_group_check=True,
                        )

            accs = [None, None]
            epi = [None, None]  # acc_sb per half
            sc_cur = qk(0)
            for s in range(NS):
                h, i = divmod(s, KT)
                if i == 0:
                    accs[h] = [
                        apsum.tile([65, 1024], F32, name=f"acc{g}", tag=f"acc{g}")
                        for g in range(2)
                    ]
                ex = exps.tile([128, 1024], BF16, name=f"ex{s % 4}", tag="ex")
                nc.scalar.activation(
                    out=ex,
                    in_=sc_cur,
                    func=mybir.ActivationFunctionType.Exp,
                    scale=SCALE,
                )
                if s + 1 < NS:
                    sc_cur = qk(s + 1)
                pv(s, ex, accs[h])
                if i == KT - 1:
                    # Accumulation for this half just finished: sum the two
                    # row-half accumulators into SBUF as bf16 (frees PSUM
                    # banks; bf16 so the transpose can run on the DMA xbar).
                    # Partitions 65-95 of acc_sb are zeroed padding so the
                    # 32-aligned xbar transpose reads defined data.
                    acc_sb = outp.tile(
                        [96, 1024], BF16, tag=f"acc_sb{h}", name=f"acc_sb{h}"
                    )
                    # Zero the 32-aligned pad range first; the copy below
                    # then overwrites partition 64 with the real data.
                    nc.vector.memset(acc_sb[64:96, :], 0.0)
                    for part in range(2):
                        sl = slice(part * 512, (part + 1) * 512)
                        nc.vector.tensor_copy(
                            out=acc_sb[0:65, sl], in_=accs[h][0][:, sl]
                        )
                    for part in range(2):
                        sl = slice(part * 512, (part + 1) * 512)
                        nc.vector.tensor_tensor(
                            acc_sb[0:65, sl],
                            acc_sb[0:65, sl],
                            accs[h][1][:, sl],
                            mybir.AluOpType.add,
                        )
                    epi[h] = acc_sb

            # Tail: transpose 128-query chunks on the DMA xbar (no PE mode
            # switch, no PSUM), then normalize; output DMAs ride both HWDGE
            # queues (SP + ACT), interleaved per half-chunk.
            for h in range(2):
                at = outp.tile([128, 8, 96], BF16, tag=f"at{h}", name=f"at{h}")
                for t in range(8):
                    nc.sync.dma_start_transpose(
                        out=at[:, t, :], in_=epi[h][:, t * 128 : (t + 1) * 128]
                    )
                ot = outp.tile([128, 8, D], F32, tag=f"ot{h}", name=f"ot{h}")
                for t in range(8):
                    rc = outp.tile([128, 1], F32, tag="rc", name=f"rc{h}_{t}")
                    nc.vector.reciprocal(rc, at[:, t, 64:65])
                    nc.vector.tensor_scalar_mul(ot[:, t, :], at[:, t, 0:D], rc)
                    if t == 3:
                        nc.sync.dma_start(
                            out=out_ap[h, :, 0 : 4 * D], in_=ot[:, 0:4, :]
                        )
                nc.scalar.dma_start(
                    out=out_ap[h, :, 4 * D : 8 * D], in_=ot[:, 4:8, :]
                )

    nc.finalize()
    return nc


_NC = None


def _get_nc():
    global _NC
    if _NC is None:
        _NC = _build()
    return _NC


def _dmajor(x, tiles):
    """[n, D] row-major -> [D, n] with column j*128+p <-> row p*tiles+j."""
    return np.ascontiguousarray(
        x.reshape(128, tiles, D).transpose(2, 1, 0).reshape(D, 128 * tiles)
    ).astype(NPBF16)


def _shard_inputs(Q1, K1, V1, Q2, K2, V2):
    """Core c handles batch c//2, modality-half c%2."""
    in_maps = []
    for c in range(NCORES):
        b, h = divmod(c, 2)
        qs = Q1[b] if h == 0 else Q2[b]
        ks = np.concatenate([K1[b], K2[b]], axis=0)
        vs = np.concatenate([V1[b], V2[b]], axis=0)
        v1 = np.ones((128, KT, D + 1), dtype=np.float32)
        v1[:, :, 0:D] = vs.reshape(128, KT, D)
        in_maps.append(
            {
                "qt": _dmajor(qs, QT),
                "kt": _dmajor(ks, KT),
                "v1": v1.reshape(128, KT * (D + 1)).astype(NPBF16),
            }
        )
    return in_maps


def _assemble(results):
    out = np.empty((B, S2, D), dtype=np.float32)
    for c in range(NCORES):
        b, h = divmod(c, 2)
        out[b, h * QSH : (h + 1) * QSH, :] = results[c]["out"]
    return out


def run(inputs, trace=False):
    nc = _get_nc()
    in_maps = _shard_inputs(
        np.asarray(inputs["Q1"]), np.asarray(inputs["K1"]), np.asarray(inputs["V1"]),
        np.asarray(inputs["Q2"]), np.asarray(inputs["K2"]), np.asarray(inputs["V2"]),
    )
    bkr = run_bass_kernel_spmd(nc, in_maps, list(range(NCORES)), trace=trace)
    return _assemble(bkr.results), bkr


def kernel(**inputs) -> np.ndarray:
    out, _ = run(inputs)
    return out


# revision 13
# speedup vs baseline: 1.3195x; 1.0047x over previous
"""Fused cross-modal attention (concat two QKV streams along sequence, full
softmax attention) on 8 Trainium2 NeuronCores.

Sharding: data-parallel over (batch b, modality-half h) -> 8 shards. Each core
computes attention for 2048 queries against the fused 4096-key sequence.

Host-side prep (per core): Q and K are pre-transposed to d-major and
pre-permuted (column j*128+p <-> row p*T+j) so the device needs no PE
transposes; V gets a ones-column appended ([V | 1], key-permuted to match K)
so the PV matmul yields the softmax denominator for free. All three are cast
to bf16 on host (full 2.4 GHz PE stream rate, half the DMA bytes).

PE row-group alternation: the PE pulls LDWEIGHTS ahead of an in-flight
matmul only when the row groups don't conflict, so every matmul here is a
64-row-contraction tile and consecutive matmuls strictly alternate between
PE row halves A (partitions 0-63) and B (64-127):
  - QK (contraction d=64): kt and qt are staged TWICE, on partitions 0-63
    and 64-127; the j0 matmul runs on half A, j1 on half B.
  - PV (contraction keys=128): split into two K=64 matmuls - V/ex rows 0-63
    on half A into accA, rows 64-127 on half B into accB. Separate PSUM
    accumulators per row half (different row tiles must not hit the same
    PSUM bank); the epilogue sums accA+accB on the DVE.
The whole main loop stays in one 64-row tiling mode (mode switches drain
the PE), so the output transposes run in the tail after the last matmul.

Pipeline per step s=(h,i): exp(s) [ACT] is emitted, then QK(s+1) [PE], then
the four PV(s) matmuls [PE] - the in-order PE computes next-tile scores
while ACT runs exp(s). Output DMA is chunked across both HWDGE queues
(SP + ACT) and issued as chunks are normalized, keeping the final drain
short.
"""

import numpy as np
import ml_dtypes

import concourse.bass as bass
import concourse.tile as tile
from concourse import mybir
from concourse.bacc import Bacc
from concourse.bass_utils import run_bass_kernel_spmd
from concourse.masks import make_identity

F32 = mybir.dt.float32
BF16 = mybir.dt.bfloat16
NPBF16 = ml_dtypes.bfloat16

B, S, D = 4, 2048, 64
S2 = 2 * S  # fused sequence length 4096
NCORES = 8
QSH = 2048  # queries per core (= S: half of the fused sequence)
KT = S2 // 128  # 32 key tiles of 128
QT = QSH // 128  # 16 query tiles of 128
SCALE = 1.0 / float(np.sqrt(D))

# kt chunk ranges (in key-tile units): first chunk small so the first
# matmul's operands land quickly.
KT_CHUNKS = [(0, 2), (2, 8), (8, 16), (16, 24), (24, 32)]
V1_CHUNKS = [(0, 4), (4, 16), (16, 32)]


def _build():
    nc = Bacc()
    # qt[d, j*128+p] = Q[p*16+j, d]; kt[d, i*128+p] = K[p*32+i, d]
    qt_d = nc.declare_dram_parameter("qt", [D, QSH], BF16, isOutput=False)
    kt_d = nc.declare_dram_parameter("kt", [D, S2], BF16, isOutput=False)
    # v1[p, i, 0:64] = V[p*32+i, :], v1[p, i, 64] = 1.0
    v1_d = nc.declare_dram_parameter("v1", [128, KT * (D + 1)], BF16, isOutput=False)
    out = nc.declare_dram_parameter("out", [QSH, D], F32, isOutput=True)

    with tile.TileContext(nc) as tc:
        with (
            tc.tile_pool(name="const", bufs=1) as const_pool,
            tc.tile_pool(name="stage", bufs=1) as stage,
            tc.tile_pool(name="psum", bufs=2, space="PSUM") as psum,
            tc.tile_pool(name="apsum", bufs=1, space="PSUM") as apsum,
            tc.tile_pool(name="exps", bufs=3) as exps,
            tc.tile_pool(name="outp", bufs=3) as outp,
        ):
            ident = const_pool.tile([128, 128], F32)
            make_identity(nc, ident)
            # Touch Exp early so the ~1.3us ACT table load overlaps the
            # input DMAs instead of stalling the first real exp.
            warm = const_pool.tile([128, 1], F32)
            nc.scalar.activation(
                out=warm, in_=ident[:, 0:1],
                func=mybir.ActivationFunctionType.Exp,
            )

            # out[p*16 + h*8 + t, :] <- half h, chunk t, partition p;
            # the 8 chunks of one half are contiguous rows per partition.
            out_ap = out[:].rearrange("(p g t) d -> g p (t d)", g=2, t=8)
            v1_ap = v1_d[:].rearrange("p (t e) -> p t e", e=D + 1)

            # kt/qt staged on BOTH partition halves (row-group alternation).
            kt_tiles = [
                stage.tile([128, (b - a) * 128], BF16, name=f"kt{a}", tag=f"kt{a}")
                for a, b in KT_CHUNKS
            ]
            qt_tiles = [
                stage.tile([128, 1024], BF16, name=f"qt{h}", tag=f"qt{h}")
                for h in range(2)
            ]
            v1_tiles = [
                stage.tile([128, b - a, D + 1], BF16, name=f"v1{a}", tag=f"v1{a}")
                for a, b in V1_CHUNKS
            ]

            def dma_dup(t, src):  # stage src into both partition halves
                nc.sync.dma_start(out=t[0:64, :], in_=src)
                nc.sync.dma_start(out=t[64:128, :], in_=src)

            a, b = KT_CHUNKS[0]
            dma_dup(kt_tiles[0], kt_d[:, a * 128 : b * 128])
            dma_dup(qt_tiles[0], qt_d[:, 0:1024])
            a, b = V1_CHUNKS[0]
            nc.sync.dma_start(out=v1_tiles[0], in_=v1_ap[:, a:b, :])
            a, b = KT_CHUNKS[1]
            dma_dup(kt_tiles[1], kt_d[:, a * 128 : b * 128])
            a, b = V1_CHUNKS[1]
            nc.sync.dma_start(out=v1_tiles[1], in_=v1_ap[:, a:b, :])
            dma_dup(qt_tiles[1], qt_d[:, 1024:2048])
            for c in range(2, len(KT_CHUNKS)):
                a, b = KT_CHUNKS[c]
                dma_dup(kt_tiles[c], kt_d[:, a * 128 : b * 128])
            a, b = V1_CHUNKS[2]
            nc.sync.dma_start(out=v1_tiles[2], in_=v1_ap[:, a:b, :])

            def kt_blk(i, half):
                for c, (a, b) in enumerate(KT_CHUNKS):
                    if a <= i < b:
                        lo = 64 * half
                        return kt_tiles[c][
                            lo : lo + 64, (i - a) * 128 : (i - a + 1) * 128
                        ]
                raise AssertionError

            def v1_blk(i, half):
                for c, (a, b) in enumerate(V1_CHUNKS):
                    if a <= i < b:
                        lo = 64 * half
                        return v1_tiles[c][lo : lo + 64, i - a, :]
                raise AssertionError

            NS = 2 * KT  # 64 pipeline steps; step s = (h, i) = divmod(s, KT)

            def qk(s):
                h, i = divmod(s, KT)
                sc = psum.tile([128, 1024], F32, tag="sc", name=f"sc{s}")
                for j in range(2):  # j0 on row half A, j1 on half B
                    lo = 64 * j
                    nc.tensor.matmul(
                        sc[:, j * 512 : (j + 1) * 512],
                        lhsT=kt_blk(i, j),
                        rhs=qt_tiles[h][lo : lo + 64, j * 512 : (j + 1) * 512],
                        start=True,
                        stop=True,
                    )
                return sc

            def pv(s, ex, acc2):
                h, i = divmod(s, KT)
                for j in range(2):
                    for half in range(2):  # key rows 0-63 on A, 64-127 on B
                        lo = 64 * half
                        nc.tensor.matmul(
                            acc2[half][:, j * 512 : (j + 1) * 512],
                            lhsT=v1_blk(i, half),
                            rhs=ex[lo : lo + 64, j * 512 : (j + 1) * 512],
                            start=(i == 0),
                            stop=(i == KT - 1),
                            skip_group_check=True,
                        )

            accs = [None, None]
            epi = [None, None]  # acc_sb per half

            # Epilogue for one half: transpose 128-query chunks on the DMA
            # xbar (no PE mode switch, no PSUM), then normalize; output DMAs
            # ride both HWDGE queues (SP + ACT), interleaved per half-chunk.
            def finish_half(h):
                at = outp.tile([128, 8, 96], BF16, tag=f"at{h}", name=f"at{h}")
                for t in range(8):
                    nc.sync.dma_start_transpose(
                        out=at[:, t, :], in_=epi[h][:, t * 128 : (t + 1) * 128]
                    )
                ot = outp.tile([128, 8, D], F32, tag=f"ot{h}", name=f"ot{h}")
                for t in range(8):
                    rc = outp.tile([128, 1], F32, tag="rc", name=f"rc{h}_{t}")
                    nc.vector.reciprocal(rc, at[:, t, 64:65])
                    nc.vector.tensor_scalar_mul(ot[:, t, :], at[:, t, 0:D], rc)
                    if t == 3:
                        nc.sync.dma_start(
                            out=out_ap[h, :, 0 : 4 * D], in_=ot[:, 0:4, :]
                        )
                nc.scalar.dma_start(
                    out=out_ap[h, :, 4 * D : 8 * D], in_=ot[:, 4:8, :]
                )

            sc_cur = qk(0)
            for s in range(NS):
                h, i = divmod(s, KT)
                if i == 0:
                    accs[h] = [
                        apsum.tile([65, 1024], F32, name=f"acc{g}", tag=f"acc{g}")
                        for g in range(2)
                    ]
                ex = exps.tile([128, 1024], BF16, name=f"ex{s % 4}", tag="ex")
                nc.scalar.activation(
                    out=ex,
                    in_=sc_cur,
                    func=mybir.ActivationFunctionType.Exp,
                    scale=SCALE,
                )
                if s + 1 < NS:
                    sc_cur = qk(s + 1)
                pv(s, ex, accs[h])
                if i == KT - 1:
                    # Accumulation for this half just finished: sum the two
                    # row-half accumulators into SBUF as bf16 (frees PSUM
                    # banks; bf16 so the transpose can run on the DMA xbar).
                    # Partitions 65-95 of acc_sb are zeroed padding so the
                    # 32-aligned xbar transpose reads defined data.
                    acc_sb = outp.tile(
                        [96, 1024], BF16, tag=f"acc_sb{h}", name=f"acc_sb{h}"
                    )
                    # Zero the 32-aligned pad range first; the copy below
                    # then overwrites partition 64 with the real data.
                    nc.vector.memset(acc_sb[64:96, :], 0.0)
                    for part in range(2):
                        sl = slice(part * 512, (part + 1) * 512)
                        nc.vector.tensor_copy(
                            out=acc_sb[0:65, sl], in_=accs[h][0][:, sl]
                        )
                    for part in range(2):
                        sl = slice(part * 512, (part + 1) * 512)
                        nc.vector.tensor_tensor(
                            acc_sb[0:65, sl],
                            acc_sb[0:65, sl],
                            accs[h][1][:, sl],
                            mybir.AluOpType.add,
                        )
                    epi[h] = acc_sb
                    # Finish this half immediately: xbar transposes, the
                    # normalize, and the output DMAs all run on DVE/DMA
                    # engines, so h=0's epilogue overlaps h=1's main loop
                    # instead of serializing into the tail.
                    finish_half(h)

    nc.finalize()
    return nc


_NC = None


def _get_nc():
    global _NC
    if _NC is None:
        _NC = _build()
    return _NC


def _dmajor(x, tiles):
    """[n, D] row-major -> [D, n] with column j*128+p <-> row p*tiles+j."""
    return np.ascontiguousarray(
        x.reshape(128, tiles, D).transpose(2, 1, 0).reshape(D, 128 * tiles)
    ).astype(NPBF16)


def _shard_inputs(Q1, K1, V1, Q2, K2, V2):
    """Core c handles batch c//2, modality-half c%2."""
    in_maps = []
    for c in range(NCORES):
        b, h = divmod(c, 2)
        qs = Q1[b] if h == 0 else Q2[b]
        ks = np.concatenate([K1[b], K2[b]], axis=0)
        vs = np.concatenate([V1[b], V2[b]], axis=0)
        v1 = np.ones((128, KT, D + 1), dtype=np.float32)
        v1[:, :, 0:D] = vs.reshape(128, KT, D)
        in_maps.append(
            {
                "qt": _dmajor(qs, QT),
                "kt": _dmajor(ks, KT),
                "v1": v1.reshape(128, KT * (D + 1)).astype(NPBF16),
            }
        )
    return in_maps


def _assemble(results):
    out = np.empty((B, S2, D), dtype=np.float32)
    for c in range(NCORES):
        b, h = divmod(c, 2)
        out[b, h * QSH : (h + 1) * QSH, :] = results[c]["out"]
    return out


def run(inputs, trace=False):
    nc = _get_nc()
    in_maps = _shard_inputs(
        np.asarray(inputs["Q1"]), np.asarray(inputs["K1"]), np.asarray(inputs["V1"]),
        np.asarray(inputs["Q2"]), np.asarray(inputs["K2"]), np.asarray(inputs["V2"]),
    )
    bkr = run_bass_kernel_spmd(nc, in_maps, list(range(NCORES)), trace=trace)
    return _assemble(bkr.results), bkr


def kernel(**inputs) -> np.ndarray:
    out, _ = run(inputs)
    return out
